# revision 12
# baseline (speedup 1.0000x reference)
"""SpMM (COO adjacency @ dense weight) on 8 Trainium2 NeuronCores.

out[r] = sum over edges (r, c) of weight[c]   (adj values are all ones)

Strategy (see sharding hint): partition edges by destination row across the
8 cores; replicate the weight table in each core's DRAM. On the host,
output rows are packed into 8*T bins of <=128 rows AND <=128 incoming
edges each (capacity-aware best-fit; in-degrees are small Poisson counts,
so exact 128-slot packing nearly always succeeds — a chunked fallback
covers the rest). Device work per 128-row output tile:
  1. one indirect DMA gathers the tile's 128 source rows of weight
     (slots pre-sorted by source column for HBM row-buffer locality),
  2. the Vector engine builds a 0/1 selection matrix S[e, r] =
     (dest[e] == r) from preloaded per-slot local-dest indices vs iota,
  3. TensorEngine matmul segment-sums psum[r, :] += S^T @ gathered,
  4. PSUM -> SBUF copy (alternating Vector/Scalar engines) and a DMA of
     the [128, 256] f32 output tile to DRAM.
Host then inverse-permutes the concatenated per-core outputs.
"""

import heapq

import numpy as np

NC_CORES = 8
P = 128
# NOTE: the runtime indirect-DMA ucode only honors [P, 1] offset APs (one
# offset per partition); multi-column offsets silently gather garbage on HW
# even though CoreSim accepts them. Keep one indirect DMA per tile chunk.
GROUP = 1  # tiles per batched indirect gather


def _build_program(n_tbl, d, t_tiles, chunks, group=GROUP):
    """Build the SPMD Bass program (identical across cores; data differs).

    chunks: per-tile slot-chunk sizes, e.g. [128] (sum = E slots/tile).
    """
    from contextlib import ExitStack

    import concourse.bacc as bacc
    import concourse.bass as bass
    import concourse.mybir as mybir
    import concourse.tile as tile

    dt = mybir.dt
    nc = bacc.Bacc(None)

    wt = nc.declare_dram_parameter("wt", [n_tbl, d], dt.float32, isOutput=False)
    colp = []
    destp = []
    for ci, cb in enumerate(chunks):
        colp.append(
            nc.declare_dram_parameter(f"cols{ci}", [cb, t_tiles], dt.int32, isOutput=False)
        )
        destp.append(
            nc.declare_dram_parameter(
                f"dest{ci}", [cb, t_tiles], dt.float32, isOutput=False
            )
        )
    iota_p = nc.declare_dram_parameter("iota", [P, P], dt.float32, isOutput=False)
    # bf16 output stream halves the HBM write traffic that competes with the
    # random-1KB gather reads; host casts back to f32 (error ~2e-3 of scale).
    out_p = nc.declare_dram_parameter("out", [t_tiles * P, d], dt.bfloat16, isOutput=True)

    with tile.TileContext(nc) as tc:
        with ExitStack() as ctx:
            cpool = ctx.enter_context(tc.tile_pool(name="const", bufs=1))
            gpools = [
                ctx.enter_context(tc.tile_pool(name=f"g{ci}", bufs=10))
                for ci in range(len(chunks))
            ]
            spools = [
                ctx.enter_context(tc.tile_pool(name=f"s{ci}", bufs=10))
                for ci in range(len(chunks))
            ]
            opool = ctx.enter_context(tc.tile_pool(name="o", bufs=10))
            pspool = ctx.enter_context(tc.tile_pool(name="ps", bufs=8, space="PSUM"))

            iota_sb = cpool.tile([P, P], dtype=dt.float32)
            nc.sync.dma_start(iota_sb[:], iota_p[:])
            cols_sb = []
            dest_sb = []
            for ci, cb in enumerate(chunks):
                ct = cpool.tile([cb, t_tiles], dtype=dt.int32, tag=f"cols{ci}")
                nc.sync.dma_start(ct[:], colp[ci][:])
                cols_sb.append(ct)
                dtile = cpool.tile([cb, t_tiles], dtype=dt.float32, tag=f"dest{ci}")
                nc.sync.dma_start(dtile[:], destp[ci][:])
                dest_sb.append(dtile)

            for g0 in range(0, t_tiles, group):
                kk = min(group, t_tiles - g0)
                gs = []
                for ci, cb in enumerate(chunks):
                    # 2-D out AP when kk == 1: the runtime indirect-DMA
                    # ucode mishandles a trailing unit middle dim.
                    shape = [cb, d] if kk == 1 else [cb, kk, d]
                    gt = gpools[ci].tile(shape, dtype=dt.float32, tag=f"g{ci}")
                    nc.gpsimd.indirect_dma_start(
                        out=gt[:],
                        out_offset=None,
                        in_=wt[:],
                        in_offset=bass.IndirectOffsetOnAxis(
                            ap=cols_sb[ci][:, g0 : g0 + kk], axis=0
                        ),
                    )
                    gs.append(gt)
                for j in range(kk):
                    t = g0 + j
                    ps = pspool.tile([P, d], dtype=dt.float32)
                    for ci, cb in enumerate(chunks):
                        s = spools[ci].tile([cb, P], dtype=dt.float32, tag=f"s{ci}")
                        nc.vector.tensor_tensor(
                            out=s[:],
                            in0=dest_sb[ci][:, t : t + 1].to_broadcast([cb, P]),
                            in1=iota_sb[:cb, :],
                            op=mybir.AluOpType.is_equal,
                        )
                        rhs = gs[ci][:] if kk == 1 else gs[ci][:, j, :]
                        nc.tensor.matmul(
                            out=ps[:],
                            lhsT=s[:],
                            rhs=rhs,
                            start=(ci == 0),
                            stop=(ci == len(chunks) - 1),
                        )
                    ot = opool.tile([P, d], dtype=dt.bfloat16)
                    if t % 2 == 0:
                        nc.vector.tensor_copy(out=ot[:], in_=ps[:])
                    else:
                        nc.scalar.copy(out=ot[:], in_=ps[:])
                    nc.sync.dma_start(out_p[t * P : (t + 1) * P, :], ot[:])

    nc.finalize()
    return nc


def _pack_bins_exact(rows, counts, nbins):
    """Best-fit pack rows into bins with <=128 slots AND <=128 rows each.

    Returns (bin_of_row, pos_of_row) or None if infeasible.
    """
    n = len(counts)
    if nbins * P < counts.sum() or counts.max() > P:
        return None
    nz = np.flatnonzero(counts)
    order = nz[np.argsort(-counts[nz], kind="stable")]
    bin_of_row = np.full(n, -1, np.int64)
    loads = np.zeros(nbins, np.int64)
    nrows = np.zeros(nbins, np.int64)
    heap = [(0, b) for b in range(nbins)]
    heapq.heapify(heap)
    for r in order.tolist():
        c = int(counts[r])
        while True:
            if not heap:
                return None
            load, b = heapq.heappop(heap)
            if load != loads[b] or nrows[b] >= P:
                continue  # stale entry or row-capacity full
            break
        if load + c > P:
            return None  # min-load bin can't fit -> nothing can
        bin_of_row[r] = b
        loads[b] += c
        nrows[b] += 1
        if loads[b] < P and nrows[b] < P:
            heapq.heappush(heap, (int(loads[b]), b))
    # zero-count rows fill the remaining row capacity anywhere
    zeros = np.flatnonzero(counts == 0)
    cap = P - nrows
    if cap.sum() < len(zeros):
        return None
    fill_bins = np.repeat(np.arange(nbins), cap)[: len(zeros)]
    bin_of_row[zeros] = fill_bins
    # positions: stable order within bin
    order_all = np.argsort(bin_of_row, kind="stable")
    bins_sorted = bin_of_row[order_all]
    starts = np.searchsorted(bins_sorted, np.arange(nbins))
    pos_of_row = np.empty(n, np.int64)
    pos_of_row[order_all] = np.arange(n, dtype=np.int64) - starts[bins_sorted]
    if pos_of_row.max() >= P:
        return None
    return bin_of_row, pos_of_row


def _pack_bins_dealt(counts, nbins):
    """Fallback: deal count-sorted rows round-robin (E may exceed 128)."""
    n = len(counts)
    order = np.argsort(-counts, kind="stable")
    idx = np.arange(n, dtype=np.int64)
    bin_of_row = np.empty(n, np.int64)
    pos_of_row = np.empty(n, np.int64)
    bin_of_row[order] = idx % nbins
    pos_of_row[order] = idx // nbins
    return bin_of_row, pos_of_row


def _prepare(adj, weight):
    """Host-side sharding: pack rows into bins, pack edges into slots."""
    w = np.ascontiguousarray(np.asarray(weight, dtype=np.float32))
    n, d = w.shape
    adj = np.asarray(adj)
    rows = adj[0].astype(np.int64)
    cols = adj[1].astype(np.int64)

    t_tiles = -(-n // (NC_CORES * P))  # ceil
    nbins = NC_CORES * t_tiles

    counts = np.bincount(rows, minlength=n)
    packed = _pack_bins_exact(rows, counts, nbins)
    if packed is None:
        packed = _pack_bins_dealt(counts, nbins)
    bin_of_row, pos_of_row = packed
    assert pos_of_row.max() < P

    # Slot assignment: edges of a bin occupy consecutive slots, ordered by
    # ascending source column — the gather's HBM reads then walk ascending
    # addresses (better row-buffer locality). Slot order within a bin is
    # free: the selection matrix maps any slot to its output row.
    eb = bin_of_row[rows]
    eo = np.lexsort((cols, eb))
    sb = eb[eo]
    starts = np.searchsorted(sb, np.arange(nbins))
    slot = np.arange(len(eo), dtype=np.int64) - starts[sb]

    e_slots = int(np.bincount(eb, minlength=nbins).max())
    if e_slots <= P:
        e_slots = P
    else:
        e_slots = max(e_slots, P + 4)
        e_slots = -4 * (-e_slots // 4)  # round up to multiple of 4

    cols_full = np.zeros((nbins, e_slots), np.int32)  # pad -> gather row 0
    dest_full = np.full((nbins, e_slots), -1.0, np.float32)  # pad: no match
    cols_full[sb, slot] = cols[eo].astype(np.int32)
    dest_full[sb, slot] = pos_of_row[rows[eo]].astype(np.float32)

    chunks = []
    rem = e_slots
    while rem > 0:
        c = min(rem, P)
        chunks.append(c)
        rem -= c

    iota = np.ascontiguousarray(
        np.broadcast_to(np.arange(P, dtype=np.float32), (P, P))
    )
    in_maps = []
    for c in range(NC_CORES):
        b0 = c * t_tiles
        b1 = b0 + t_tiles
        m = {"wt": w, "iota": iota}
        off = 0
        for ci, cb in enumerate(chunks):
            m[f"cols{ci}"] = np.ascontiguousarray(cols_full[b0:b1, off : off + cb].T)
            m[f"dest{ci}"] = np.ascontiguousarray(dest_full[b0:b1, off : off + cb].T)
            off += cb
        in_maps.append(m)

    meta = {
        "n": n,
        "d": d,
        "t_tiles": t_tiles,
        "chunks": chunks,
        "bin_of_row": bin_of_row,
        "pos_of_row": pos_of_row,
    }
    return in_maps, meta


LAST_RESULT = None


def kernel(adj, size, weight):
    global LAST_RESULT
    from concourse.bass_utils import run_bass_kernel_spmd

    in_maps, meta = _prepare(adj, weight)
    nc = _build_program(meta["n"], meta["d"], meta["t_tiles"], meta["chunks"])
    res = run_bass_kernel_spmd(nc, in_maps, core_ids=list(range(NC_CORES)))
    LAST_RESULT = res
    flat = np.concatenate([r["out"] for r in res.results], axis=0).astype(np.float32)
    return flat[meta["bin_of_row"] * P + meta["pos_of_row"]]


# revision 14
# speedup vs baseline: 1.0083x; 1.0083x over previous
"""SpMM (COO adjacency @ dense weight) on 8 Trainium2 NeuronCores.

out[r] = sum over edges (r, c) of weight[c]   (adj values are all ones)

Strategy (see sharding hint): partition edges by destination row across the
8 cores; replicate the weight table in each core's DRAM. On the host,
output rows are packed into 8*T bins of <=128 rows AND <=128 incoming
edges each (capacity-aware best-fit; in-degrees are small Poisson counts,
so exact 128-slot packing nearly always succeeds — a chunked fallback
covers the rest). Device work per 128-row output tile:
  1. one indirect DMA gathers the tile's 128 source rows of weight
     (slots pre-sorted by source column for HBM row-buffer locality),
  2. the Vector engine builds a 0/1 selection matrix S[e, r] =
     (dest[e] == r) from preloaded per-slot local-dest indices vs iota,
  3. TensorEngine matmul segment-sums psum[r, :] += S^T @ gathered,
  4. PSUM -> SBUF copy (alternating Vector/Scalar engines) and a DMA of
     the [128, 256] f32 output tile to DRAM.
Host then inverse-permutes the concatenated per-core outputs.
"""

import heapq

import numpy as np

NC_CORES = 8
P = 128
# NOTE: the runtime indirect-DMA ucode only honors [P, 1] offset APs (one
# offset per partition); multi-column offsets silently gather garbage on HW
# even though CoreSim accepts them. Keep one indirect DMA per tile chunk.
GROUP = 1  # tiles per batched indirect gather


def _build_program(n_tbl, d, t_tiles, chunks, group=GROUP):
    """Build the SPMD Bass program (identical across cores; data differs).

    chunks: per-tile slot-chunk sizes, e.g. [128] (sum = E slots/tile).
    """
    from contextlib import ExitStack

    import concourse.bacc as bacc
    import concourse.bass as bass
    import concourse.mybir as mybir
    import concourse.tile as tile

    dt = mybir.dt
    # Two SWDGE queues: alternate indirect gathers between qPoolDynamic and
    # qPoolDynamic1 so Q7 descriptor generation can overlap across contexts.
    nc = bacc.Bacc(None, num_swdge_queues=2)

    wt = nc.declare_dram_parameter("wt", [n_tbl, d], dt.float32, isOutput=False)
    colp = []
    destp = []
    for ci, cb in enumerate(chunks):
        colp.append(
            nc.declare_dram_parameter(f"cols{ci}", [cb, t_tiles], dt.int32, isOutput=False)
        )
        destp.append(
            nc.declare_dram_parameter(
                f"dest{ci}", [cb, t_tiles], dt.float32, isOutput=False
            )
        )
    iota_p = nc.declare_dram_parameter("iota", [P, P], dt.float32, isOutput=False)
    out_p = nc.declare_dram_parameter("out", [t_tiles * P, d], dt.float32, isOutput=True)

    with tile.TileContext(nc) as tc:
        with ExitStack() as ctx:
            cpool = ctx.enter_context(tc.tile_pool(name="const", bufs=1))
            gpools = [
                ctx.enter_context(tc.tile_pool(name=f"g{ci}", bufs=10))
                for ci in range(len(chunks))
            ]
            spools = [
                ctx.enter_context(tc.tile_pool(name=f"s{ci}", bufs=10))
                for ci in range(len(chunks))
            ]
            opool = ctx.enter_context(tc.tile_pool(name="o", bufs=10))
            pspool = ctx.enter_context(tc.tile_pool(name="ps", bufs=8, space="PSUM"))

            iota_sb = cpool.tile([P, P], dtype=dt.float32)
            nc.sync.dma_start(iota_sb[:], iota_p[:])
            cols_sb = []
            dest_sb = []
            for ci, cb in enumerate(chunks):
                ct = cpool.tile([cb, t_tiles], dtype=dt.int32, tag=f"cols{ci}")
                nc.sync.dma_start(ct[:], colp[ci][:])
                cols_sb.append(ct)
                dtile = cpool.tile([cb, t_tiles], dtype=dt.float32, tag=f"dest{ci}")
                nc.sync.dma_start(dtile[:], destp[ci][:])
                dest_sb.append(dtile)

            for g0 in range(0, t_tiles, group):
                kk = min(group, t_tiles - g0)
                gs = []
                for ci, cb in enumerate(chunks):
                    # 2-D out AP when kk == 1: the runtime indirect-DMA
                    # ucode mishandles a trailing unit middle dim.
                    shape = [cb, d] if kk == 1 else [cb, kk, d]
                    gt = gpools[ci].tile(shape, dtype=dt.float32, tag=f"g{ci}")
                    gins = nc.gpsimd.indirect_dma_start(
                        out=gt[:],
                        out_offset=None,
                        in_=wt[:],
                        in_offset=bass.IndirectOffsetOnAxis(
                            ap=cols_sb[ci][:, g0 : g0 + kk], axis=0
                        ),
                    )
                    if g0 % 2 == 1:
                        gins.ins.queue = "qPoolDynamic1"
                    gs.append(gt)
                for j in range(kk):
                    t = g0 + j
                    ps = pspool.tile([P, d], dtype=dt.float32)
                    for ci, cb in enumerate(chunks):
                        s = spools[ci].tile([cb, P], dtype=dt.float32, tag=f"s{ci}")
                        nc.vector.tensor_tensor(
                            out=s[:],
                            in0=dest_sb[ci][:, t : t + 1].to_broadcast([cb, P]),
                            in1=iota_sb[:cb, :],
                            op=mybir.AluOpType.is_equal,
                        )
                        rhs = gs[ci][:] if kk == 1 else gs[ci][:, j, :]
                        nc.tensor.matmul(
                            out=ps[:],
                            lhsT=s[:],
                            rhs=rhs,
                            start=(ci == 0),
                            stop=(ci == len(chunks) - 1),
                        )
                    ot = opool.tile([P, d], dtype=dt.float32)
                    if t % 2 == 0:
                        nc.vector.tensor_copy(out=ot[:], in_=ps[:])
                    else:
                        nc.scalar.copy(out=ot[:], in_=ps[:])
                    nc.sync.dma_start(out_p[t * P : (t + 1) * P, :], ot[:])

    nc.finalize()
    return nc


def _pack_bins_exact(rows, counts, nbins):
    """Best-fit pack rows into bins with <=128 slots AND <=128 rows each.

    Returns (bin_of_row, pos_of_row) or None if infeasible.
    """
    n = len(counts)
    if nbins * P < counts.sum() or counts.max() > P:
        return None
    nz = np.flatnonzero(counts)
    order = nz[np.argsort(-counts[nz], kind="stable")]
    bin_of_row = np.full(n, -1, np.int64)
    loads = np.zeros(nbins, np.int64)
    nrows = np.zeros(nbins, np.int64)
    heap = [(0, b) for b in range(nbins)]
    heapq.heapify(heap)
    for r in order.tolist():
        c = int(counts[r])
        while True:
            if not heap:
                return None
            load, b = heapq.heappop(heap)
            if load != loads[b] or nrows[b] >= P:
                continue  # stale entry or row-capacity full
            break
        if load + c > P:
            return None  # min-load bin can't fit -> nothing can
        bin_of_row[r] = b
        loads[b] += c
        nrows[b] += 1
        if loads[b] < P and nrows[b] < P:
            heapq.heappush(heap, (int(loads[b]), b))
    # zero-count rows fill the remaining row capacity anywhere
    zeros = np.flatnonzero(counts == 0)
    cap = P - nrows
    if cap.sum() < len(zeros):
        return None
    fill_bins = np.repeat(np.arange(nbins), cap)[: len(zeros)]
    bin_of_row[zeros] = fill_bins
    # positions: stable order within bin
    order_all = np.argsort(bin_of_row, kind="stable")
    bins_sorted = bin_of_row[order_all]
    starts = np.searchsorted(bins_sorted, np.arange(nbins))
    pos_of_row = np.empty(n, np.int64)
    pos_of_row[order_all] = np.arange(n, dtype=np.int64) - starts[bins_sorted]
    if pos_of_row.max() >= P:
        return None
    return bin_of_row, pos_of_row


def _pack_bins_dealt(counts, nbins):
    """Fallback: deal count-sorted rows round-robin (E may exceed 128)."""
    n = len(counts)
    order = np.argsort(-counts, kind="stable")
    idx = np.arange(n, dtype=np.int64)
    bin_of_row = np.empty(n, np.int64)
    pos_of_row = np.empty(n, np.int64)
    bin_of_row[order] = idx % nbins
    pos_of_row[order] = idx // nbins
    return bin_of_row, pos_of_row


def _prepare(adj, weight):
    """Host-side sharding: pack rows into bins, pack edges into slots."""
    w = np.ascontiguousarray(np.asarray(weight, dtype=np.float32))
    n, d = w.shape
    adj = np.asarray(adj)
    rows = adj[0].astype(np.int64)
    cols = adj[1].astype(np.int64)

    t_tiles = -(-n // (NC_CORES * P))  # ceil
    nbins = NC_CORES * t_tiles

    counts = np.bincount(rows, minlength=n)
    packed = _pack_bins_exact(rows, counts, nbins)
    if packed is None:
        packed = _pack_bins_dealt(counts, nbins)
    bin_of_row, pos_of_row = packed
    assert pos_of_row.max() < P

    # Slot assignment: edges of a bin occupy consecutive slots, ordered by
    # ascending source column — the gather's HBM reads then walk ascending
    # addresses (better row-buffer locality). Slot order within a bin is
    # free: the selection matrix maps any slot to its output row.
    eb = bin_of_row[rows]
    eo = np.lexsort((cols, eb))
    sb = eb[eo]
    starts = np.searchsorted(sb, np.arange(nbins))
    slot = np.arange(len(eo), dtype=np.int64) - starts[sb]

    e_slots = int(np.bincount(eb, minlength=nbins).max())
    if e_slots <= P:
        e_slots = P
    else:
        e_slots = max(e_slots, P + 4)
        e_slots = -4 * (-e_slots // 4)  # round up to multiple of 4

    cols_full = np.zeros((nbins, e_slots), np.int32)  # pad -> gather row 0
    dest_full = np.full((nbins, e_slots), -1.0, np.float32)  # pad: no match
    cols_full[sb, slot] = cols[eo].astype(np.int32)
    dest_full[sb, slot] = pos_of_row[rows[eo]].astype(np.float32)

    chunks = []
    rem = e_slots
    while rem > 0:
        c = min(rem, P)
        chunks.append(c)
        rem -= c

    iota = np.ascontiguousarray(
        np.broadcast_to(np.arange(P, dtype=np.float32), (P, P))
    )
    in_maps = []
    for c in range(NC_CORES):
        b0 = c * t_tiles
        b1 = b0 + t_tiles
        m = {"wt": w, "iota": iota}
        off = 0
        for ci, cb in enumerate(chunks):
            m[f"cols{ci}"] = np.ascontiguousarray(cols_full[b0:b1, off : off + cb].T)
            m[f"dest{ci}"] = np.ascontiguousarray(dest_full[b0:b1, off : off + cb].T)
            off += cb
        in_maps.append(m)

    meta = {
        "n": n,
        "d": d,
        "t_tiles": t_tiles,
        "chunks": chunks,
        "bin_of_row": bin_of_row,
        "pos_of_row": pos_of_row,
    }
    return in_maps, meta


LAST_RESULT = None


def kernel(adj, size, weight):
    global LAST_RESULT
    from concourse.bass_utils import run_bass_kernel_spmd

    in_maps, meta = _prepare(adj, weight)
    nc = _build_program(meta["n"], meta["d"], meta["t_tiles"], meta["chunks"])
    res = run_bass_kernel_spmd(nc, in_maps, core_ids=list(range(NC_CORES)))
    LAST_RESULT = res
    flat = np.concatenate([r["out"] for r in res.results], axis=0)
    return flat[meta["bin_of_row"] * P + meta["pos_of_row"]]
